# revision 2
# baseline (speedup 1.0000x reference)
"""AutoCorrelation layer kernel v3 for 8 Trainium2 NeuronCores.

Math: the reference's rfft/irfft over the zero-padded head dim collapses to
corr[b,l] = (1/(H*L)) * sum_h (q[b,l]@WqS + bqS)_h * (k[b,l]@WkS + bkS)_h.
Downstream: top-6, softmax, weighted v-gather, agg = vbar@Wv + bv,
out = agg @ Wp + bp.

v3 vs v2: AllToAll(mesh) instead of ring AllGather for the tiny agg
exchange; bias handled with host-expanded bp (no per-chunk gpsimd chain);
balanced DVE/ACT/gpsimd evacuation; exchange DMAs off the wp queue.
"""
import sys

sys.path.insert(0, "/opt/trn_rl_repo")

import numpy as np
import ml_dtypes
import concourse.bass as bass
import concourse.mybir as mybir
import concourse.tile as tile
from concourse import bacc
from concourse.bass_utils import run_bass_kernel_spmd
from concourse.masks import make_identity

F32 = mybir.dt.float32
F16 = mybir.dt.float16
BF16 = mybir.dt.bfloat16
FP8 = mybir.dt.float8e3

N_CORES = 8
B, L, D, H, DK = 8, 1024, 256, 8, 32
K_TOP = 6
NSH = (L * D) // N_CORES          # 32768 output cols per core
SCALE = 1.0 / (H * L)
WPSCALE = 24.0

WPJ = NSH // 2048                 # 16 wp loads of [128, 2, 2048]
NDC = NSH // 4096                 # 8 double-chunks of output

XCHG = "a2a"                      # "a2a" | "ags" | "ag"
TRACE = False
LAST_RESULT = None
_CACHE = {}


def _build_nc():
    nc = bacc.Bacc("TRN2", target_bir_lowering=False, debug=False, num_devices=N_CORES)

    qt_d = nc.dram_tensor("qt", [D, L], F32, kind="ExternalInput").ap()
    kt_d = nc.dram_tensor("kt", [D, L], F32, kind="ExternalInput").ap()
    v_d = nc.dram_tensor("v", [128, 8, D], BF16, kind="ExternalInput").ap()
    wqs_d = nc.dram_tensor("wqs", [D, H], F32, kind="ExternalInput").ap()
    wks_d = nc.dram_tensor("wks", [D, H], F32, kind="ExternalInput").ap()
    bqk_d = nc.dram_tensor("bqk", [1, 16], F32, kind="ExternalInput").ap()
    wv_d = nc.dram_tensor("wv", [D, D], F16, kind="ExternalInput").ap()
    bv_d = nc.dram_tensor("bv", [128, 2], F32, kind="ExternalInput").ap()
    wp_d = nc.dram_tensor("wp", [D, NSH], FP8, kind="ExternalInput").ap()
    bp8_d = nc.dram_tensor("bp8", [B, NSH], BF16, kind="ExternalInput").ap()
    out_d = nc.dram_tensor("out", [B, NSH], BF16, kind="ExternalOutput").ap()

    with tile.TileContext(nc) as tc:
        with (
            tc.tile_pool(name="cst", bufs=1) as cst,
            tc.tile_pool(name="wpp", bufs=WPJ) as wpp,
            tc.tile_pool(name="outp", bufs=3) as outp,
            tc.tile_pool(name="ps_pre", bufs=3, space="PSUM") as ps_pre,
            tc.tile_pool(name="ps_out", bufs=4, space="PSUM") as ps_out,
        ):
            # ---------- phase 0: DMAs ----------
            # critical-path inputs first on the scalar queue
            qt_sb = cst.tile([128, 2, L], F32)
            nc.scalar.dma_start(qt_sb[:, :, :], qt_d.rearrange("(c p) l -> p c l", p=128))
            kt_sb = cst.tile([128, 2, L], F32)
            nc.scalar.dma_start(kt_sb[:, :, :], kt_d.rearrange("(c p) l -> p c l", p=128))
            wqs_sb = cst.tile([128, 2, H], F32)
            nc.scalar.dma_start(wqs_sb[:, :, :], wqs_d.rearrange("(c p) h -> p c h", p=128))
            wks_sb = cst.tile([128, 2, H], F32)
            nc.scalar.dma_start(wks_sb[:, :, :], wks_d.rearrange("(c p) h -> p c h", p=128))
            bqk_sb = cst.tile([1, 16], F32)
            nc.scalar.dma_start(bqk_sb[:, :], bqk_d)
            wv_sb = cst.tile([128, 2, D], F16)
            nc.scalar.dma_start(wv_sb[:, :, :], wv_d.rearrange("(c p) d -> p c d", p=128))
            bv_sb = cst.tile([128, 2], F32)
            nc.scalar.dma_start(bv_sb[:, :], bv_d)
            v_sb = cst.tile([128, 8, D], BF16)
            nc.scalar.dma_start(v_sb[:, :, :], v_d)
            bp8_sb = cst.tile([8, NSH], BF16)
            nc.scalar.dma_start(bp8_sb[:, :], bp8_d)

            # wp stream alone on the sync queue: 16 x 512KB
            wp_sb = []
            for j in range(WPJ):
                w = wpp.tile([128, 2, 2048], FP8, tag="wp")
                nc.sync.dma_start(
                    w[:, :, :],
                    wp_d[:, 2048 * j:2048 * (j + 1)].rearrange("(c p) n -> p c n", p=128))
                wp_sb.append(w)

            # ---------- constants ----------
            ident8 = cst.tile([8, 8], F32)
            make_identity(nc, ident8[:, :])
            ident128 = cst.tile([128, 128], F32)
            make_identity(nc, ident128[:, :])
            one1 = cst.tile([1, 1], F32)
            nc.vector.memset(one1[:, :], 1.0)
            ones128v = cst.tile([1, 128], F32)
            nc.vector.memset(ones128v[:, :], 1.0)
            sones = cst.tile([8, 1], F32)
            nc.vector.memset(sones[:, :], SCALE)
            wpc = cst.tile([1, 1], F32)
            nc.vector.memset(wpc[:, :], 1.0 / WPSCALE)

            bqv_ps = ps_pre.tile([8, 2], F32, tag="pre")
            nc.tensor.matmul(bqv_ps[:, 0:1], bqk_sb[:, 0:8], one1[:, :], start=True, stop=True)
            nc.tensor.matmul(bqv_ps[:, 1:2], bqk_sb[:, 8:16], one1[:, :], start=True, stop=True)
            bqv = cst.tile([8, 2], F32)
            nc.vector.tensor_copy(bqv[:, :], bqv_ps[:, :])

            # ---------- preprocessing (this core's batch) ----------
            qs = cst.tile([8, L], F32)
            ks = cst.tile([8, L], F32)
            for (src, wsum, bcol, dst) in ((qt_sb, wqs_sb, 0, qs), (kt_sb, wks_sb, 1, ks)):
                for half in range(2):
                    sl = slice(512 * half, 512 * (half + 1))
                    px = ps_pre.tile([8, 512], F32, tag="pre")
                    nc.tensor.matmul(px[:, :], wsum[:, 0, :], src[:, 0, sl], start=True, stop=False)
                    nc.tensor.matmul(px[:, :], wsum[:, 1, :], src[:, 1, sl], start=False, stop=True)
                    nc.vector.tensor_scalar(
                        out=dst[:, sl], in0=px[:, :],
                        scalar1=bqv[:, bcol:bcol + 1], scalar2=None, op0=mybir.AluOpType.add)

            nc.vector.tensor_mul(qs[:, :], qs[:, :], ks[:, :])

            corrT = cst.tile([128, 8], F32)
            for t in range(8):
                ct = ps_pre.tile([128, 1], F32, tag="pre")
                nc.tensor.matmul(ct[:, :], qs[:, 128 * t:128 * (t + 1)], sones[:, :],
                                 start=True, stop=True)
                nc.vector.tensor_copy(corrT[:, t:t + 1], ct[:, :])

            c8_ps = ps_pre.tile([8, 128], F32, tag="pre")
            nc.tensor.transpose(c8_ps[:, :], corrT[:, :], ident128[:, :])
            corr8 = cst.tile([8, 128], F32)
            nc.vector.tensor_copy(corr8[:, :], c8_ps[:, :])
            t88 = cst.tile([8, 8], F32)
            nc.vector.max(t88[:, :], corr8[:, :])
            c64_ps = ps_pre.tile([1, 64], F32, tag="pre")
            for jj in range(8):
                nc.tensor.matmul(c64_ps[:, 8 * jj:8 * (jj + 1)], ident8[:, jj:jj + 1],
                                 t88[:, :], start=True, stop=True)
            c64 = cst.tile([1, 64], F32)
            nc.vector.tensor_copy(c64[:, :], c64_ps[:, :])
            top8 = cst.tile([1, 8], F32)
            nc.vector.max(top8[:, :], c64[:, :])

            negm = cst.tile([1, 1], F32)
            nc.vector.tensor_scalar_mul(negm[:, :], top8[:, 0:1], -1.0)
            e6 = cst.tile([1, K_TOP], F32)
            nc.scalar.activation(e6[:, :], top8[:, 0:K_TOP],
                                 mybir.ActivationFunctionType.Exp,
                                 bias=negm[:, 0:1], scale=1.0)
            z6 = cst.tile([1, 1], F32)
            nc.vector.reduce_sum(out=z6[:, :], in_=e6[:, :], axis=mybir.AxisListType.X)
            zinv = cst.tile([1, 1], F32)
            nc.vector.reciprocal(zinv[:, :], z6[:, :])
            zs = cst.tile([1, 1], F32)
            nc.vector.tensor_mul(zs[:, :], zinv[:, :], wpc[:, :])

            nt = cst.tile([1, 2], F32)
            nc.vector.tensor_copy(nt[:, 0:1], negm[:, :])
            nc.vector.tensor_copy(nt[:, 1:2], top8[:, 5:6])
            b128_ps = ps_pre.tile([128, 3], F32, tag="pre")
            nc.tensor.matmul(b128_ps[:, 0:2], ones128v[:, :], nt[:, :], start=True, stop=True)
            nc.tensor.matmul(b128_ps[:, 2:3], ones128v[:, :], zs[:, :], start=True, stop=True)
            b128 = cst.tile([128, 3], F32)
            nc.vector.tensor_copy(b128[:, :], b128_ps[:, :])

            e8T = cst.tile([128, 8], F32)
            nc.scalar.activation(e8T[:, :], corrT[:, :],
                                 mybir.ActivationFunctionType.Exp,
                                 bias=b128[:, 0:1], scale=1.0)
            m8T = cst.tile([128, 8], F32)
            nc.vector.tensor_scalar(out=m8T[:, :], in0=corrT[:, :],
                                    scalar1=b128[:, 1:2], scalar2=None,
                                    op0=mybir.AluOpType.is_ge)
            selT = cst.tile([128, 8], F16)
            nc.vector.tensor_mul(selT[:, :], e8T[:, :], m8T[:, :])

            vbarT = cst.tile([128, 2], F16)
            for m in range(2):
                pv = ps_pre.tile([128, 1], F32, tag="pre")
                for t in range(8):
                    nc.tensor.matmul(pv[:, :], v_sb[:, t, 128 * m:128 * (m + 1)],
                                     selT[:, t:t + 1], start=(t == 0), stop=(t == 7))
                nc.vector.tensor_copy(vbarT[:, m:m + 1], pv[:, :])

            agg_sb = cst.tile([128, 2], F32)
            for m in range(2):
                pa = ps_pre.tile([128, 1], F32, tag="pre")
                nc.tensor.matmul(pa[:, :], wv_sb[:, 0, 128 * m:128 * (m + 1)],
                                 vbarT[:, 0:1], start=True, stop=False)
                nc.tensor.matmul(pa[:, :], wv_sb[:, 1, 128 * m:128 * (m + 1)],
                                 vbarT[:, 1:2], start=False, stop=True)
                nc.vector.tensor_scalar(
                    out=agg_sb[:, m:m + 1], in0=pa[:, :],
                    scalar1=b128[:, 2:3], scalar2=bv_sb[:, m:m + 1],
                    op0=mybir.AluOpType.mult, op1=mybir.AluOpType.add)

            # agg as one row [1, 256] via two PE transposes of [128, 1] columns
            agg_row = cst.tile([1, D], F32)
            for m in range(2):
                tr = ps_pre.tile([1, 128], F32, tag="pre")
                nc.tensor.transpose(tr[:, :], agg_sb[:, m:m + 1], ident128[:, :])
                nc.vector.tensor_copy(agg_row[:, 128 * m:128 * (m + 1)], tr[:, :])

            # ---------- exchange ----------
            if XCHG == "a2a":
                # replicate own agg row to 8 partitions, AllToAll scatters row j
                # to rank j -> result rows are rank-ordered agg vectors
                agg_rep = cst.tile([8, D], F32)
                nc.gpsimd.partition_broadcast(agg_rep[:, :], agg_row[:, :])
                a2a_in = nc.dram_tensor("a2a_in", [B, D], F32).ap()
                nc.gpsimd.dma_start(a2a_in, agg_rep[:, :])
                a2a_out = nc.dram_tensor("a2a_out", [B, D], F32).ap()
                nc.gpsimd.collective_compute(
                    "AllToAll", mybir.AluOpType.bypass,
                    replica_groups=[list(range(N_CORES))],
                    ins=[a2a_in.opt()], outs=[a2a_out.opt()])
                agf = cst.tile([8, D], F32)
                nc.gpsimd.dma_start(agf[:, :], a2a_out)
            elif XCHG == "ags":
                agg_in = nc.dram_tensor("agg_in", [1, D], F32).ap()
                nc.gpsimd.dma_start(agg_in, agg_row[:, :])
                agg_all = nc.dram_tensor("agg_all", [B, D], F32, addr_space="Shared").ap()
                nc.gpsimd.collective_compute(
                    "AllGather", mybir.AluOpType.bypass,
                    replica_groups=[list(range(N_CORES))],
                    ins=[agg_in.opt()], outs=[agg_all.opt()])
                agf = cst.tile([8, D], F32)
                nc.gpsimd.dma_start(agf[:, :], agg_all)
            else:
                agg_in = nc.dram_tensor("agg_in", [1, D], F32).ap()
                nc.gpsimd.dma_start(agg_in, agg_row[:, :])
                agg_all = nc.dram_tensor("agg_all", [B, D], F32).ap()
                nc.gpsimd.collective_compute(
                    "AllGather", mybir.AluOpType.bypass,
                    replica_groups=[list(range(N_CORES))],
                    ins=[agg_in.opt()], outs=[agg_all.opt()])
                agf = cst.tile([8, D], F32)
                nc.gpsimd.dma_start(agf[:, :], agg_all)

            aggT = cst.tile([128, 2, 8], F16)
            for c in range(2):
                at_ps = ps_pre.tile([128, 8], F32, tag="pre")
                nc.tensor.transpose(at_ps[:, :], agf[:, 128 * c:128 * (c + 1)], ident8[:, :])
                nc.vector.tensor_copy(aggT[:, c, :], at_ps[:, :])

            # ---------- big projection ----------
            # 8 double-chunks of 4096 cols; per dc: tiles 0-1 DVE-fused evac,
            # tiles 2-3 ACT copy (+ one [8,2048] bias add on DVE/gpsimd alt.)
            for dc in range(NDC):
                dcs = slice(4096 * dc, 4096 * (dc + 1))
                och = outp.tile([8, 4096], BF16, tag="oc")
                for s in range(8):
                    j = 8 * dc + s
                    jj, js = j // 4, j % 4
                    ps = ps_out.tile([8, 512], F32, tag="po")
                    nc.tensor.matmul(ps[:, :], aggT[:, 0, :],
                                     wp_sb[jj][:, 0, 512 * js:512 * (js + 1)],
                                     start=True, stop=False)
                    nc.tensor.matmul(ps[:, :], aggT[:, 1, :],
                                     wp_sb[jj][:, 1, 512 * js:512 * (js + 1)],
                                     start=False, stop=True)
                    osl = slice(512 * s, 512 * (s + 1))
                    gsl = slice(4096 * dc + 512 * s, 4096 * dc + 512 * (s + 1))
                    if s < 4:
                        nc.vector.tensor_tensor(
                            out=och[:, osl], in0=ps[:, :], in1=bp8_sb[:, gsl],
                            op=mybir.AluOpType.add)
                    else:
                        nc.scalar.copy(och[:, osl], ps[:, :])
                # bias for the ACT-copied half, alternating DVE / gpsimd
                half = slice(2048, 4096)
                bsl = slice(4096 * dc + 2048, 4096 * (dc + 1))
                eng = nc.vector if dc % 2 == 0 else nc.gpsimd
                eng.tensor_tensor(out=och[:, half], in0=och[:, half],
                                  in1=bp8_sb[:, bsl], op=mybir.AluOpType.add)
                nc.scalar.dma_start(out_d[:, dcs], och[:, :])

    nc.finalize()
    return nc


def _get_nc():
    if "nc" not in _CACHE:
        _CACHE["nc"] = _build_nc()
    return _CACHE["nc"]


def _host_prep(queries, keys, values, Wq, bq, Wk, bk, Wv, bv, Wp, bp):
    queries = np.asarray(queries, np.float32)
    keys = np.asarray(keys, np.float32)
    values = np.asarray(values, np.float32)
    Wq = np.asarray(Wq, np.float32)
    Wk = np.asarray(Wk, np.float32)
    Wv = np.asarray(Wv, np.float32)
    bq = np.asarray(bq, np.float32)
    bk = np.asarray(bk, np.float32)
    bv = np.asarray(bv, np.float32)
    Wp = np.asarray(Wp, np.float32)
    bp = np.asarray(bp, np.float32)

    WqS = np.ascontiguousarray(Wq.reshape(D, H, DK).sum(-1))
    WkS = np.ascontiguousarray(Wk.reshape(D, H, DK).sum(-1))
    bqS = bq.reshape(H, DK).sum(-1)
    bkS = bk.reshape(H, DK).sum(-1)
    bqk = np.concatenate([bqS, bkS]).reshape(1, 16).astype(np.float32)
    wv16 = np.ascontiguousarray(Wv.astype(np.float16))
    bv2 = np.ascontiguousarray((bv / WPSCALE).reshape(2, 128).T.astype(np.float32))

    wp8_full = (Wp * WPSCALE).astype(ml_dtypes.float8_e3m4)
    bp_bf = bp.astype(ml_dtypes.bfloat16)

    in_maps = []
    for i in range(N_CORES):
        cols = slice(NSH * i, NSH * (i + 1))
        m = {
            "qt": np.ascontiguousarray(queries[i].T),
            "kt": np.ascontiguousarray(keys[i].T),
            "v": np.ascontiguousarray(
                values[i].reshape(8, 128, D).transpose(1, 0, 2).astype(ml_dtypes.bfloat16)),
            "wqs": WqS, "wks": WkS, "bqk": bqk,
            "wv": wv16, "bv": bv2,
            "wp": np.ascontiguousarray(wp8_full[:, cols]),
            "bp8": np.ascontiguousarray(np.broadcast_to(bp_bf[cols], (B, NSH))),
        }
        in_maps.append(m)
    return in_maps


def kernel(queries, keys, values, Wq, bq, Wk, bk, Wv, bv, Wp, bp):
    in_maps = _host_prep(queries, keys, values, Wq, bq, Wk, bk, Wv, bv, Wp, bp)
    nc = _get_nc()
    res = run_bass_kernel_spmd(nc, in_maps, core_ids=list(range(N_CORES)), trace=TRACE)
    global LAST_RESULT
    LAST_RESULT = res
    out = np.concatenate(
        [res.results[i]["out"].astype(np.float32) for i in range(N_CORES)], axis=1)
    return out.reshape(B, L, D)
